# revision 33
# baseline (speedup 1.0000x reference)
"""Causal dilated conv1d (K=3, dilation=2, N=128 channels) on Trainium2.

out[b,t,i] = sum_{j,k} x[b, t-2k, j] * weight[i,j,k] + bias[i]

Strategy (8-core SPMD, pure data parallel over batch; bf16 datapath):
  - each core handles 4 of the 32 batch rows; weight/bias replicated.
  - host interleaves the core's 4 rows with a 16-row zero head:
    x4[tt, q, j] = x[b_q, tt-16, j]  (zeros for tt<16).  One DMA xbar
    transpose per slab loads [chunk+16, 4*128] DRAM directly as
    [128(j), 4(q), chunk+16(t)] in SBUF (transposed row r = q*128+j
    lands partition-first), so the PE does no transpose-in and the zero
    head doubles as causal pad.  All transposes stay on the sync queue
    (two concurrent xbar streams corrupt data; >5 in-flight transposes
    stall the sequencer) with one independent strip tile per slab.
  - taps: 3 accumulated bf16 matmuls per 512-wide PSUM window, moving
    operand = strip shifted by 2k columns.
  - bias is added during the PSUM->SBUF copy, alternating DVE/ACT.
  - HYBRID output path: rows q0/q1 are transposed back to [t, i] on
    device (delayed PE identity-matmuls + DVE copies — this work also
    keeps the PE busy across slab boundaries where the 2-deep xbar
    pipeline would otherwise stall it); rows q2/q3 are stored in [i, t]
    layout and the host restores them during the gather.  Both paths
    use 128 large contiguous store descriptors.
  - stores: SWDGE on the idle GpSimd queue, except the last two slabs'
    q2/q3 stores which use the by-then-idle sync queue so the tail
    drains right behind the last bias-add.
  - variable slab sizes (1024, 2048x3, 1024): small first slab starts
    the PE earlier, small last slab drains the tail faster.
  - output is bf16; host upconverts to fp32.
"""

import threading

import numpy as np
import ml_dtypes

import concourse.bass as bass  # noqa: F401  (bass types used via bacc/tile)
import concourse.mybir as mybir
import concourse.tile as tile
from concourse import bacc
from concourse.bass_utils import run_bass_kernel_spmd
from concourse.masks import make_identity

P = 128
KTAPS = 3
DIL = 2
NCORES = 8
B_FULL, T_FULL = 32, 8192
B_CORE = B_FULL // NCORES  # 4
HEAD = 16  # zero rows prepended on host (causal pad + keeps slabs aligned)
QDEV = 2  # rows restored to [t, i] on device; the rest fixed on host

FP32 = mybir.dt.float32
BF16 = mybir.dt.bfloat16


def build(T=T_FULL, slabs=(1024, 2048, 2048, 2048, 1024)):
    """Build the per-core Bass module. Same NEFF runs SPMD on all 8 cores."""
    assert sum(slabs) == T
    nc = bacc.Bacc(
        "TRN2",
        target_bir_lowering=False,
        debug=False,
        enable_asserts=False,
        num_devices=NCORES,
    )
    Q = B_CORE  # interleaved batch rows per core
    x_d = nc.dram_tensor("x", [HEAD + T, Q, P], BF16, kind="ExternalInput")
    w_d = nc.dram_tensor("w", [P, KTAPS * P], BF16, kind="ExternalInput")
    b_d = nc.dram_tensor("b", [P, 1], FP32, kind="ExternalInput")
    # q0/q1: natural [t, i] layout; q2/q3: transposed [i, t], host restores
    on_d = nc.dram_tensor("on", [QDEV, T, P], BF16, kind="ExternalOutput")
    ot_d = nc.dram_tensor("ot", [Q - QDEV, P, T], BF16, kind="ExternalOutput")

    x_ap, on_ap, ot_ap = x_d.ap(), on_d.ap(), ot_d.ap()
    SW = 512  # tap-matmul moving width (1 PSUM bank of fp32)
    n_slabs = len(slabs)
    starts = [sum(slabs[:i]) for i in range(n_slabs)]
    maxc = max(slabs)

    with tile.TileContext(nc) as tc:
        with (
            tc.tile_pool(name="const", bufs=1) as cp,
            tc.tile_pool(name="strip", bufs=n_slabs) as sp,
            tc.tile_pool(name="oT", bufs=8) as otp,
            tc.tile_pool(name="oc", bufs=4) as ocp,
            tc.tile_pool(name="pacc", bufs=5, space="PSUM") as paccp,
            tc.tile_pool(name="pto", bufs=3, space="PSUM") as ptop,
        ):
            ident = cp.tile([P, P], FP32)
            make_identity(nc, ident)
            ident_bf = cp.tile([P, P], BF16)
            nc.vector.tensor_copy(ident_bf[:], ident[:])
            w_sb = cp.tile([P, KTAPS * P], BF16)
            nc.sync.dma_start(w_sb[:], w_d.ap())
            bias_sb = cp.tile([P, 1], FP32)
            nc.sync.dma_start(bias_sb[:], b_d.ap())

            strips = [
                sp.tile([P, Q * (maxc + HEAD)], BF16, tag="strip", name=f"strip{i}")
                for i in range(n_slabs)
            ]

            # delayed device-side restore of q0/q1 windows: interleaving the
            # identity-matmul transposes behind the NEXT units' taps keeps
            # the PE dense across slab boundaries.
            pending = []  # fifo of (t0, chunk, q, oT)

            def emit_tout(t0_p, ch_p, q_p, oT_p):
                # column r + R*p of the window holds t = t0 + p*R + r
                oTv = oT_p.rearrange("n (p r) -> n r p", p=P)
                oc_full = ocp.tile([P, maxc], BF16, tag="oc")
                oc = oc_full[:, :ch_p]
                for g in range(ch_p // SW):
                    pto = ptop.tile([P, SW], BF16, tag="pto")
                    for rr in range(4):
                        r = g * 4 + rr
                        nc.tensor.transpose(
                            pto[:, rr * P : (rr + 1) * P], oTv[:, r, :], ident_bf
                        )
                    nc.vector.tensor_copy(oc[:, g * SW : (g + 1) * SW], pto[:])
                nc.gpsimd.dma_start(
                    on_ap[q_p, t0_p : t0_p + ch_p, :].rearrange(
                        "(p r) i -> p (r i)", p=P
                    ),
                    oc,
                )

            n_add = 0
            for c, chunk in enumerate(slabs):
                SLABT = chunk + HEAD
                nc.sync.dma_start_transpose(
                    strips[c][:, : Q * SLABT].rearrange("p (q t) -> p q t", q=Q),
                    x_ap[starts[c] : starts[c] + SLABT, :, :],
                )
                for q in range(Q):
                    base = q * SLABT + HEAD
                    oT_full = otp.tile([P, maxc], BF16, tag="oT")
                    oT = oT_full[:, :chunk]
                    for s in range(chunk // SW):
                        pacc = paccp.tile([P, SW], FP32, tag="pacc")
                        for k in range(KTAPS):
                            off = base + s * SW - DIL * k
                            nc.tensor.matmul(
                                pacc[:],
                                w_sb[:, k * P : (k + 1) * P],
                                strips[c][:, off : off + SW],
                                start=(k == 0),
                                stop=(k == KTAPS - 1),
                            )
                        # bias during PSUM->SBUF copy, DVE/ACT alternating
                        dst = oT[:, s * SW : (s + 1) * SW]
                        if n_add % 2 == 0:
                            nc.vector.tensor_scalar_add(dst, pacc[:], bias_sb[:])
                        else:
                            nc.scalar.add(dst, pacc[:], bias_sb[:])
                        n_add += 1
                    if q < QDEV:
                        pending.append((starts[c], chunk, q, oT))
                    else:
                        # direct [i, t] store; tail slabs use the idle sync
                        # queue so the drain follows the last add closely
                        store_eng = nc.sync if c >= n_slabs - 2 else nc.gpsimd
                        store_eng.dma_start(
                            ot_ap[q - QDEV, :, starts[c] : starts[c] + chunk], oT
                        )
                    # one delayed restore per q-step keeps PE dense without
                    # letting the backlog grow
                    if len(pending) >= 3:
                        emit_tout(*pending.pop(0))
            for args in pending:
                emit_tout(*args)
    nc.compile()
    return nc


_cache = {}
_lock = threading.Lock()


def _get_nc():
    with _lock:
        if "nc" not in _cache:
            _cache["nc"] = build()
        return _cache["nc"]


def prep_inputs(x, weight, bias):
    # w_all[j, k*128 + i] = weight[i, j, k]
    w_all = np.ascontiguousarray(
        np.transpose(np.asarray(weight, np.float32), (1, 2, 0))
        .reshape(P, KTAPS * P)
        .astype(ml_dtypes.bfloat16)
    )
    b2 = np.ascontiguousarray(np.asarray(bias, np.float32).reshape(P, 1))
    xb = np.asarray(x, np.float32).astype(ml_dtypes.bfloat16)
    # per core: x4[tt, q, j] = x[b_q, tt-HEAD, j], 16 zero rows at the top
    # (q-major: the xbar fills transposed rows partition-first, so row
    # r = q*128+j lands at partition j, sub-slab q)
    xi = np.zeros((NCORES, HEAD + T_FULL, B_CORE, P), dtype=ml_dtypes.bfloat16)
    xg = xb.reshape(NCORES, B_CORE, T_FULL, P)
    xi[:, HEAD:, :, :] = np.swapaxes(xg, 1, 2)
    return xi, w_all, b2


def kernel(x, weight, bias, _trace=False):
    xi, w_all, b2 = prep_inputs(x, weight, bias)
    nc = _get_nc()
    in_maps = [
        {"x": np.ascontiguousarray(xi[c]), "w": w_all, "b": b2}
        for c in range(NCORES)
    ]
    res = run_bass_kernel_spmd(nc, in_maps, core_ids=list(range(NCORES)), trace=_trace)
    # q0/q1 come back [QDEV, T, P]; q2/q3 come back [Q-QDEV, P, T]
    parts = []
    for r in res.results:
        on = np.asarray(r["on"]).astype(np.float32)
        ot = np.swapaxes(np.asarray(r["ot"]), 1, 2).astype(np.float32)
        parts.append(np.concatenate([on, ot], axis=0))
    out = np.concatenate(parts, axis=0)
    if _trace:
        kernel.last_results = res
    return out


# revision 34
# speedup vs baseline: 1.0758x; 1.0758x over previous
"""Causal dilated conv1d (K=3, dilation=2, N=128 channels) on Trainium2.

out[b,t,i] = sum_{j,k} x[b, t-2k, j] * weight[i,j,k] + bias[i]

Strategy (8-core SPMD, pure data parallel over batch; bf16 datapath):
  - each core handles 4 of the 32 batch rows; weight/bias replicated.
  - host interleaves the core's 4 rows with a 16-row zero head:
    x4[tt, q, j] = x[b_q, tt-16, j]  (zeros for tt<16).  A DMA xbar
    transpose loads each [chunk+16, 512] DRAM slab directly as
    [128(j), 4(q), chunk+16(t)] in SBUF (transposed row r = q*128+j
    lands partition-first: partition j, sub-slab q), so one transfer
    feeds all 4 rows' strips, the PE does no transpose-in, and the
    zero head doubles as causal pad (no edge special-casing).
  - per-slab strip tiles (fresh pool tile per slab) keep the slab
    transposes free of false WAR deps: the tile framework tracks
    hazards at tile granularity, not byte ranges.
  - variable slab sizes (1024, 2048x3, 1024): small first slab starts
    the PE ~5us earlier, small last slab drains the tail faster.
  - taps: 3 accumulated bf16 matmuls per 512-wide PSUM window, moving
    operand = strip shifted by 2k columns.
  - ACT adds per-partition bias while copying PSUM->SBUF (bf16 out).
  - PE transposes the [i,t] result back to [t,i] using a stride-16
    moving operand so each output partition holds 16 consecutive t rows
    -> 4 KB contiguous store descriptors, issued on the idle GpSimd
    SWDGE queue so stores never block the xbar transpose stream.
  - output is bf16; host upconverts to fp32.
"""

import threading

import numpy as np
import ml_dtypes

import concourse.bass as bass  # noqa: F401  (bass types used via bacc/tile)
import concourse.mybir as mybir
import concourse.tile as tile
from concourse import bacc
from concourse.bass_utils import run_bass_kernel_spmd
from concourse.masks import make_identity

P = 128
KTAPS = 3
DIL = 2
NCORES = 8
B_FULL, T_FULL = 32, 8192
B_CORE = B_FULL // NCORES  # 4
HEAD = 16  # zero rows prepended on host (causal pad + keeps slabs aligned)

FP32 = mybir.dt.float32
BF16 = mybir.dt.bfloat16


def build(T=T_FULL, slabs=(1024, 2048, 2048, 2048, 1024)):
    """Build the per-core Bass module. Same NEFF runs SPMD on all 8 cores.

    Variable slab sizes: small first slab so the PE starts after ~2.5us
    of xbar streaming instead of ~9.5, small last slab so the restore/
    store pipeline drains quickly.
    """
    assert sum(slabs) == T
    nc = bacc.Bacc(
        "TRN2",
        target_bir_lowering=False,
        debug=False,
        enable_asserts=False,
        num_devices=NCORES,
    )
    Q = B_CORE  # interleaved batch rows per core
    x_d = nc.dram_tensor("x", [HEAD + T, Q, P], BF16, kind="ExternalInput")
    w_d = nc.dram_tensor("w", [P, KTAPS * P], BF16, kind="ExternalInput")
    b_d = nc.dram_tensor("b", [P, 1], FP32, kind="ExternalInput")
    o_d = nc.dram_tensor("o", [Q, T, P], BF16, kind="ExternalOutput")

    x_ap, o_ap = x_d.ap(), o_d.ap()
    SW = 512  # tap-matmul moving width (1 PSUM bank of fp32)

    with tile.TileContext(nc) as tc:
        with (
            tc.tile_pool(name="const", bufs=1) as cp,
            tc.tile_pool(name="strip", bufs=5) as sp,
            tc.tile_pool(name="oT", bufs=2) as otp,
            tc.tile_pool(name="oc", bufs=6) as ocp,
            tc.tile_pool(name="pacc", bufs=4, space="PSUM") as paccp,
            tc.tile_pool(name="pto", bufs=4, space="PSUM") as ptop,
        ):
            ident = cp.tile([P, P], FP32)
            make_identity(nc, ident)
            ident_bf = cp.tile([P, P], BF16)
            nc.vector.tensor_copy(ident_bf[:], ident[:])
            w_sb = cp.tile([P, KTAPS * P], BF16)
            nc.sync.dma_start(w_sb[:], w_d.ap())
            bias_sb = cp.tile([P, 1], FP32)
            nc.sync.dma_start(bias_sb[:], b_d.ap())

            # one-(slab,row)-delayed transpose-out so the PE never stalls
            # waiting on ACT's PSUM->SBUF bias copies.
            pending = []  # fifo of (t0, chunk, q, oT)

            def emit_tout(t0_p, ch_p, q_p, oT_p):
                # column r + R*p of the row-q window holds t = t0 + p*R + r
                oTv = oT_p[:, q_p * ch_p : (q_p + 1) * ch_p].rearrange(
                    "n (p r) -> n r p", p=P
                )
                oc_full = ocp.tile([P, max(slabs)], BF16, tag="oc")
                oc = oc_full[:, :ch_p]
                for g in range(ch_p // SW):
                    pto = ptop.tile([P, SW], BF16, tag="pto")
                    for rr in range(4):
                        r = g * 4 + rr
                        nc.tensor.transpose(
                            pto[:, rr * P : (rr + 1) * P], oTv[:, r, :], ident_bf
                        )
                    nc.vector.tensor_copy(oc[:, g * SW : (g + 1) * SW], pto[:])
                # SWDGE on the idle GpSimd queue: stores never block the sync
                # queue's xbar transpose stream.
                nc.gpsimd.dma_start(
                    o_ap[q_p, t0_p : t0_p + ch_p, :].rearrange(
                        "(p r) i -> p (r i)", p=P
                    ),
                    oc[:],
                )

            t0 = 0
            for chunk in slabs:
                SLABT = chunk + HEAD
                # strip[j, q*SLABT + tt] = x[b_q, t0 + tt - HEAD, j].
                # A fresh tile per slab: no WAR/RAW aliasing between slabs,
                # so all slab transposes stream back-to-back on the xbar.
                strip_full = sp.tile([P, Q * (max(slabs) + HEAD)], BF16, tag="strip")
                strip = strip_full[:, : Q * SLABT]
                # xbar-transposed load of one slab, all 4 rows at once:
                # [chunk+16, 4*128] DRAM -> [128, 4, chunk+16] SBUF.
                nc.sync.dma_start_transpose(
                    strip.rearrange("p (q t) -> p q t", q=Q),
                    x_ap[t0 : t0 + SLABT, :, :],
                )
                oT_full = otp.tile([P, Q * max(slabs)], BF16, tag="oT")
                oT = oT_full[:, : Q * chunk]
                for q in range(Q):
                    base = q * SLABT + HEAD
                    for s in range(chunk // SW):
                        pacc = paccp.tile([P, SW], FP32, tag="pacc")
                        for k in range(KTAPS):
                            off = base + s * SW - DIL * k
                            nc.tensor.matmul(
                                pacc[:],
                                w_sb[:, k * P : (k + 1) * P],
                                strip[:, off : off + SW],
                                start=(k == 0),
                                stop=(k == KTAPS - 1),
                            )
                        nc.scalar.add(
                            oT[:, q * chunk + s * SW : q * chunk + (s + 1) * SW],
                            pacc[:],
                            bias_sb[:],
                        )
                    # delayed transpose-out: restore one pending (slab,row)
                    # from 2 rows ago while taps keep the PE dense.
                    if len(pending) >= 2:
                        emit_tout(*pending.pop(0))
                    pending.append((t0, chunk, q, oT))
                t0 += chunk
            for args in pending:
                emit_tout(*args)
    nc.compile()
    return nc


_cache = {}
_lock = threading.Lock()


def _get_nc():
    with _lock:
        if "nc" not in _cache:
            _cache["nc"] = build()
        return _cache["nc"]


def prep_inputs(x, weight, bias):
    # w_all[j, k*128 + i] = weight[i, j, k]
    w_all = np.ascontiguousarray(
        np.transpose(np.asarray(weight, np.float32), (1, 2, 0))
        .reshape(P, KTAPS * P)
        .astype(ml_dtypes.bfloat16)
    )
    b2 = np.ascontiguousarray(np.asarray(bias, np.float32).reshape(P, 1))
    xb = np.asarray(x, np.float32).astype(ml_dtypes.bfloat16)
    # per core: x4[tt, q, j] = x[b_q, tt-HEAD, j], 16 zero rows at the top
    # (q-major: the xbar fills transposed rows partition-first, so row
    # r = q*128+j lands at partition j, sub-slab q)
    xi = np.zeros((NCORES, HEAD + T_FULL, B_CORE, P), dtype=ml_dtypes.bfloat16)
    xg = xb.reshape(NCORES, B_CORE, T_FULL, P)
    xi[:, HEAD:, :, :] = np.swapaxes(xg, 1, 2)
    return xi, w_all, b2


def kernel(x, weight, bias, _trace=False):
    xi, w_all, b2 = prep_inputs(x, weight, bias)
    nc = _get_nc()
    in_maps = [
        {"x": np.ascontiguousarray(xi[c]), "w": w_all, "b": b2}
        for c in range(NCORES)
    ]
    res = run_bass_kernel_spmd(nc, in_maps, core_ids=list(range(NCORES)), trace=_trace)
    out = np.concatenate(
        [np.asarray(r["o"]).astype(np.float32) for r in res.results], axis=0
    )
    if _trace:
        kernel.last_results = res
    return out
